# revision 19
# baseline (speedup 1.0000x reference)
"""Trainium2 Bass kernel for nn_MultiHeadCrossAttention (v2).

Sharding: 8 cores = 4 batches x 2 head-groups (8 heads each).

v2 over baseline: softmax exp is split between ACT (table exp) and DVE
(bit-trick fast-exp producing bf16 bit patterns via int16 tensor_scalar),
attention av-matmuls are pipelined *within* each (pair, c2) unit at lag-2
behind the score matmuls (tiny et footprint, no PE stall on exp drain),
qkproj bias-adds run on ACT (Identity+bias), normalize multiplies run on
GPSIMD, and staging DMAs are ordered weights-first.
"""

import sys

sys.path.insert(0, "/opt/trn_rl_repo")

import numpy as np
import ml_dtypes
from contextlib import ExitStack

import concourse.bass as bass
import concourse.bacc as bacc
import concourse.mybir as mybir
from concourse.tile import TileContext

DIM = 1024
H = 16
HD = 64
ROT = 32
B = 4
QL = 2048
KL = 2048
G = 2                # head-group (tensor-parallel) factor
HL = H // G          # 8 local heads
DL = HL * HD         # 512 local feature dims
NPAIR = HL // 2      # 4 head pairs
NCORE = 8

F32 = mybir.dt.float32
BF16 = mybir.dt.bfloat16
I16 = mybir.dt.int16
AFT = mybir.ActivationFunctionType
ALU = mybir.AluOpType
bf16 = ml_dtypes.bfloat16

# fast-exp: bf16 bits of exp(0.125*s) ~= round(s*FE_A + FE_B) as int16
FE_A = float(0.125 * np.log2(np.e) * 128.0)
FE_B = 16256.0

# exp engine split by half-tile index (j*4 + h*2 + n) % 16: 5/16 on DVE
DVE_HALF = {1, 4, 7, 10, 13}

_NC_CACHE = {}


def _rot_patterns():
    """cc/ss blend patterns [128, QL] for the stream-shuffle rotary."""
    inv_freq = 1.0 / (10000.0 ** (np.arange(0, ROT, 2, dtype=np.float64) / ROT))
    t = np.arange(QL, dtype=np.float64)
    freqs = t[:, None] * inv_freq[None, :]          # [QL, 16]
    cos_p = np.ones((HD, QL), np.float64)
    sin_p = np.zeros((HD, QL), np.float64)
    for d in range(ROT):
        j = d // 2
        cos_p[d] = np.cos(freqs[:, j])
        sin_p[d] = np.sin(freqs[:, j]) * (-1.0 if d % 2 == 0 else 1.0)
    cc = np.tile(cos_p, (2, 1)).astype(np.float32)  # [128, QL]
    ss = np.tile(sin_p, (2, 1)).astype(np.float32)
    return cc, ss


def _build_nc():
    if "nc" in _NC_CACHE:
        return _NC_CACHE["nc"]
    nc = bacc.Bacc("TRN2", target_bir_lowering=False)

    d = {}
    for name, shape, dt in [
        ("qT", [DIM, QL], BF16), ("kT", [DIM, KL], BF16), ("vT", [DIM, KL], BF16),
        ("wqT", [DIM, DL], BF16), ("wkT", [DIM, DL], BF16), ("wvT", [DIM, DL], BF16),
        ("woT", [DL, DIM], BF16),
        ("bqp", [128, NPAIR], F32), ("bkp", [128, NPAIR], F32),
        ("bv", [1, DL], BF16), ("ones1", [1, 128], BF16),
        ("cc", [128, QL], BF16), ("ss", [128, QL], BF16),
    ]:
        d[name] = nc.dram_tensor(name, shape, dt, kind="ExternalInput")
    out_d = nc.dram_tensor("out", [QL, DIM], F32, kind="ExternalOutput")

    qT_t = d["qT"].rearrange("(a p) n -> a p n", p=128)     # [8, 128, QL]
    kT_t = d["kT"].rearrange("(a p) n -> a p n", p=128)
    vT_t = d["vT"].rearrange("(a p) n -> a p n", p=128)
    wqT_t = d["wqT"].rearrange("(a p) n -> a p n", p=128)   # [8, 128, DL]
    wkT_t = d["wkT"].rearrange("(a p) n -> a p n", p=128)
    wvT_t = d["wvT"].rearrange("(a p) n -> a p n", p=128)
    woT_t = d["woT"].rearrange("(a p) n -> a p n", p=128)   # [4, 128, DIM]
    out_t = out_d.rearrange("(a p) n -> a p n", p=128)      # [16, 128, DIM]

    SWAP_MASK = [(j + 1 if j % 2 == 0 else j - 1) for j in range(32)]

    with TileContext(nc) as tc, ExitStack() as top:
        # ---------------- constants & weights (DMA'd first) ----------------
        def alt(a):
            return nc.sync if a % 2 == 0 else nc.gpsimd

        # DMA issue order = consumption order: wk+bk -> kT (halves, both
        # queues) -> rotary consts -> v weights+data -> q weights+data -> wo.
        consts = top.enter_context(tc.tile_pool(name="consts", bufs=1))
        bk_s = consts.tile([128, NPAIR], F32)
        nc.sync.dma_start(out=bk_s, in_=d["bkp"][:, :])
        bq_s = consts.tile([128, NPAIR], F32)
        nc.gpsimd.dma_start(out=bq_s, in_=d["bqp"][:, :])
        bv_s = consts.tile([1, DL], BF16)
        nc.gpsimd.dma_start(out=bv_s, in_=d["bv"][:, :])
        ones_s = consts.tile([1, 128], BF16)
        nc.gpsimd.dma_start(out=ones_s, in_=d["ones1"][:, :])

        wpool = top.enter_context(tc.tile_pool(name="wpool", bufs=1))
        wks = [wpool.tile([128, DL], BF16, name=f"wks{i}") for i in range(8)]
        for a in range(8):
            alt(a).dma_start(out=wks[a], in_=wkT_t[a])

        etp = top.enter_context(tc.tile_pool(name="etp", bufs=16))
        stage = top.enter_context(tc.tile_pool(name="stage", bufs=8))
        ks = [stage.tile([128, KL], BF16, tag="stage", name=f"ks{i}") for i in range(8)]
        for half in range(2):
            for a in range(8):
                alt(a).dma_start(out=ks[a][:, half * 1024:(half + 1) * 1024],
                                 in_=kT_t[a][:, half * 1024:(half + 1) * 1024])

        cc_s = consts.tile([128, QL], BF16)
        nc.gpsimd.dma_start(out=cc_s, in_=d["cc"][:, :])
        ss_s = consts.tile([128, QL], BF16)
        nc.sync.dma_start(out=ss_s, in_=d["ss"][:, :])

        wvs = [wpool.tile([128, DL], BF16, name=f"wvs{i}") for i in range(8)]
        for a in range(8):
            alt(a).dma_start(out=wvs[a], in_=wvT_t[a])
        ve = [etp.tile([128, 1024], BF16, tag="et", name=f"ve{i}") for i in range(8)]
        for a in range(8):
            alt(a).dma_start(out=ve[a], in_=vT_t[a][:, 0:1024])
        wqs = [wpool.tile([128, DL], BF16, name=f"wqs{i}") for i in range(8)]
        for a in range(8):
            alt(a).dma_start(out=wqs[a], in_=wqT_t[a])
        qe = [etp.tile([128, 1024], BF16, tag="et", name=f"qe{i}") for i in range(8)]
        for a in range(8):
            alt(a).dma_start(out=qe[a], in_=qT_t[a][:, 0:1024])
        vs = [stage.tile([128, 1024], BF16, tag="stage", name=f"vs{i}") for i in range(8)]
        for a in range(8):
            alt(a).dma_start(out=vs[a], in_=vT_t[a][:, 1024:2048])
        qs = [stage.tile([128, QL], BF16, tag="stage", name=f"qs{i}") for i in range(8)]
        for a in range(8):
            alt(a).dma_start(out=qs[a], in_=qT_t[a])
        wo_s = [wpool.tile([128, DIM], BF16, name=f"wo{i}") for i in range(NPAIR)]
        for i in range(NPAIR):
            alt(i).dma_start(out=wo_s[i], in_=woT_t[i])

        # Warm the ACT exp table early.
        warm = consts.tile([1, 8], F32)
        nc.scalar.activation(out=warm, in_=ones_s[0:1, 0:8], func=AFT.Exp)

        # ---------------- persistent activations ----------------
        qh_pool = top.enter_context(tc.tile_pool(name="qh", bufs=1))
        qhT = [qh_pool.tile([128, QL], BF16, name=f"qh{i}") for i in range(NPAIR)]
        kh_pool = top.enter_context(tc.tile_pool(name="kh", bufs=1))
        khT = [kh_pool.tile([128, KL], BF16, name=f"kh{i}") for i in range(NPAIR)]
        vh_pool = top.enter_context(tc.tile_pool(name="vh", bufs=1))
        # Per kl-tile: 4 pairs x [vh_even(64) | 1 | 1 | vh_odd(64)] bf16.
        vh = [vh_pool.tile([128, NPAIR * 130], BF16, name=f"vh{i}") for i in range(16)]
        at_pool = top.enter_context(tc.tile_pool(name="atn", bufs=1))
        apT = [at_pool.tile([128, QL], BF16, name=f"apT{i}") for i in range(NPAIR)]

        # ---------------- working pools ----------------
        rtmp = top.enter_context(tc.tile_pool(name="rtmp", bufs=1))
        aup = top.enter_context(tc.tile_pool(name="aup", bufs=4))
        btp = top.enter_context(tc.tile_pool(name="btp", bufs=4))
        rcp = top.enter_context(tc.tile_pool(name="rcp", bufs=6))
        outst = top.enter_context(tc.tile_pool(name="outst", bufs=3))
        dscr = top.enter_context(tc.tile_pool(name="dscr", bufs=8, space="DRAM"))
        psS = top.enter_context(tc.tile_pool(name="psS", bufs=4, space="PSUM"))
        psP = top.enter_context(tc.tile_pool(name="psP", bufs=4, space="PSUM"))

        # ---------------- helpers ----------------
        def rotary(dst, mt):
            for c2 in range(2):
                cs = slice(c2 * 1024, (c2 + 1) * 1024)
                sw = rtmp.tile([128, 1024], BF16, tag="sw")
                nc.vector.stream_shuffle(out=sw, in_=dst[mt][:, cs], mask=SWAP_MASK)
                t1 = rtmp.tile([128, 1024], BF16, tag="t1")
                nc.vector.tensor_tensor(out=t1, in0=sw, in1=ss_s[:, cs], op=ALU.mult)
                t2 = rtmp.tile([128, 1024], BF16, tag="t2")
                nc.vector.tensor_tensor(out=t2, in0=dst[mt][:, cs], in1=cc_s[:, cs], op=ALU.mult)
                nc.vector.tensor_tensor(out=dst[mt][:, cs], in0=t1, in1=t2, op=ALU.add)

        def k_rhs(a, c2, n):
            return ks[a][:, c2 * 1024 + n * 512:c2 * 1024 + (n + 1) * 512]

        def q0_rhs(a, c2, n):
            # c2=0 half lives in the early tiles, c2=1 in the stage ring
            if c2 == 0:
                return qe[a][:, n * 512:(n + 1) * 512]
            return qs[a][:, 1024 + n * 512:1024 + (n + 1) * 512]

        def q_rhs(a, c2, n):
            return qs[a][:, c2 * 1024 + n * 512:c2 * 1024 + (n + 1) * 512]

        def qkproj_chunk(rhs_fn, ws, b_s, dst, mt, c2, n, pool, ptag):
            """One [128, 512] chunk of a q/k projection + ACT bias-add."""
            ps = pool.tile([128, 512], F32, tag=ptag, name=f"pj{mt}{c2}{n}")
            for a in range(8):
                nc.tensor.matmul(
                    ps,
                    lhsT=ws[a][:, mt * 128:(mt + 1) * 128],
                    rhs=rhs_fn(a, c2, n),
                    start=(a == 0), stop=(a == 7),
                )
            nc.scalar.activation(
                out=dst[mt][:, c2 * 1024 + n * 512:c2 * 1024 + (n + 1) * 512],
                in_=ps, func=AFT.Identity, bias=b_s[:, mt:mt + 1])

        def vproj_tile(t):
            ps = psS.tile([128, DL], F32, tag="S", name=f"vp{t}")
            for a in range(8):
                lhs = (ve[a][:, t * 128:(t + 1) * 128] if t < 8
                       else vs[a][:, (t - 8) * 128:(t - 7) * 128])
                nc.tensor.matmul(ps, lhsT=lhs, rhs=wvs[a],
                                 start=(a == 0), stop=False)
            nc.tensor.matmul(ps, lhsT=ones_s, rhs=bv_s, start=False, stop=True)
            vtr = vh[t].rearrange("p (g h e) -> p g h e", h=2, e=65)
            nc.gpsimd.memset(vtr[:, :, :, 64:65], 1.0)
            psr = ps.rearrange("p (g h e) -> p g h e", h=2, e=64)
            nc.scalar.activation(out=vtr[:, :, :, 0:64], in_=psr, func=AFT.Copy)

        def outproj_chunk(qt, dc):
            ps = psS.tile([128, 512], F32, tag="S", name=f"op{qt}{dc}")
            for pi in range(NPAIR):
                nc.tensor.matmul(
                    ps,
                    lhsT=apT[pi][:, qt * 128:(qt + 1) * 128],
                    rhs=wo_s[pi][:, dc * 512:(dc + 1) * 512],
                    start=(pi == 0), stop=(pi == NPAIR - 1),
                )
            ot = outst.tile([128, 512], F32, tag="o")
            if dc == 0:
                nc.vector.tensor_copy(out=ot, in_=ps)
            else:
                nc.scalar.activation(out=ot, in_=ps, func=AFT.Copy)
            eng = nc.sync if qt % 2 == 0 else nc.gpsimd
            eng.dma_start(out=out_t[qt][:, dc * 512:(dc + 1) * 512], in_=ot)

        # ---------------- phase A: kproj, vproj, qproj(0) ----------------
        for mt in range(NPAIR):
            for c2 in range(2):
                for n in range(2):
                    qkproj_chunk(k_rhs, wks, bk_s, khT, mt, c2, n, psP, "P")
            rotary(khT, mt)
        for t in range(16):
            vproj_tile(t)
        for mt_c2_n in range(4):
            c2, n = divmod(mt_c2_n, 2)
            qkproj_chunk(q0_rhs, wqs, bq_s, qhT, 0, c2, n, psP, "P")
        rotary(qhT, 0)

        # ---------------- phase B: 8 units, continuous attn pipeline ----
        LAG = 2
        ustate = {}   # ug -> dict(p, c2, ets, accs)

        def attn_advance(g):
            """Emit the attn accumulation step for global position g, plus the
            unit drain (denominators, normalize) when its last step lands."""
            if g < 0 or g >= 128:
                return
            ug, tg = divmod(g, 16)
            st = ustate[ug]
            p, c2 = st["p"], st["c2"]
            if tg == 0:
                st["accs"] = {
                    (h, n): psP.tile([128, 512], F32, tag="P",
                                     name=f"pa{ug}{h}{n}")
                    for h in range(2) for n in range(2)}
            accs = st["accs"]
            for h in range(2):
                lhs = vh[tg][:, p * 130 + h * 65: p * 130 + (h + 1) * 65]
                for n in range(2):
                    nc.tensor.matmul(
                        accs[(h, n)][0:65, :],
                        lhsT=lhs,
                        rhs=st["ets"][(tg, h)][:, n * 512:(n + 1) * 512],
                        start=(tg == 0), stop=(tg == 15),
                    )
            if tg < 15:
                return
            # drain: copy out, reciprocal of the denominator row in place,
            # partition-broadcast it, normalize on gpsimd
            for h in range(2):
                for n in range(2):
                    atu = aup.tile([128, 512], F32, tag="atu")
                    if (h + n) % 2 == 0:
                        nc.vector.tensor_copy(out=atu[0:65, :],
                                              in_=accs[(h, n)][0:65, :])
                    else:
                        nc.scalar.activation(out=atu[0:65, :],
                                             in_=accs[(h, n)][0:65, :],
                                             func=AFT.Copy)
                    nc.vector.reciprocal(out=atu[64:65, :], in_=atu[64:65, :])
                    ds = dscr.tile([1, 512], F32, tag="dsc")
                    nc.sync.dma_start(out=ds, in_=atu[64:65, :])
                    bt = btp.tile([64, 512], F32, tag="bc")
                    nc.sync.dma_start(out=bt, in_=ds[0:1, :].to_broadcast([64, 512]))
                    nc.gpsimd.tensor_tensor(
                        out=apT[p][h * 64:(h + 1) * 64,
                                   c2 * 1024 + n * 512:c2 * 1024 + (n + 1) * 512],
                        in0=atu[0:64, :], in1=bt, op=ALU.mult)
            del ustate[ug]

        def emit_unit(u):
            p, c2 = divmod(u, 2)
            ets = {}
            ustate[u] = {"p": p, "c2": c2, "ets": ets, "accs": None}

            for j in range(16):
                # h-alternating emission: adjacent score MMs use disjoint
                # row groups so the PE can co-execute them
                etj = {}
                for h in range(2):
                    etj[h] = etp.tile([128, 1024], BF16, tag="et",
                                      name=f"et{u}{j}{h}")
                    ets[(j, h)] = etj[h]
                for n in range(2):
                    for h in range(2):
                        ps = psS.tile([128, 512], F32, tag="S", name=f"s{u}{j}{h}{n}")
                        nc.tensor.matmul(
                            ps,
                            lhsT=khT[p][h * 64:(h + 1) * 64,
                                        j * 128:(j + 1) * 128],
                            rhs=qhT[p][h * 64:(h + 1) * 64,
                                       c2 * 1024 + n * 512:
                                       c2 * 1024 + (n + 1) * 512],
                            start=True, stop=True,
                            tile_position=(h * 64, 0),
                        )
                        if (j * 4 + h * 2 + n) % 16 in DVE_HALF:
                            nc.vector.tensor_scalar(
                                out=etj[h].bitcast(I16)[:, n * 512:(n + 1) * 512],
                                in0=ps, scalar1=FE_A, scalar2=FE_B,
                                op0=ALU.mult, op1=ALU.add)
                        else:
                            nc.scalar.activation(
                                out=etj[h][:, n * 512:(n + 1) * 512], in_=ps,
                                func=AFT.Exp, scale=0.125)
                attn_advance(16 * u + j - LAG)
                if u == 7 and j >= 5:
                    # unit 6's apT c2=0 normalize lands ~iter 1-4; 11 chunks
                    # here, the remaining 21 in the tail
                    ci = j - 5
                    outproj_chunk(ci // 2, ci % 2)
                # filler: qproj pair 1 in unit 1, pair 2 in units 2-3,
                # pair 3 in units 4-5 (1 chunk per 4 or 8 iters); rotary at
                # j=15 so it overlaps the unit edge.
                if u == 1 and j % 4 == 1:
                    c2f, nf = divmod(j // 4, 2)
                    qkproj_chunk(q_rhs, wqs, bq_s, qhT, 1, c2f, nf, psS, "S")
                elif u in (2, 3, 4, 5) and j % 8 == 1:
                    idx = (u - 2) * 2 + j // 8
                    pair = 2 + idx // 4
                    c2f, nf = divmod(idx % 4, 2)
                    qkproj_chunk(q_rhs, wqs, bq_s, qhT, pair, c2f, nf, psS, "S")
                if j == 15 and u in (1, 3, 5):
                    rotary(qhT, (u + 1) // 2)

        for u in range(8):
            emit_unit(u)
        for g in range(128 - LAG, 128):
            attn_advance(g)

        # ---------------- tail: remaining out-projection ----------------
        for ci in range(11, 32):
            outproj_chunk(ci // 2, ci % 2)

    nc.compile()
    _NC_CACHE["nc"] = nc
    return nc


def _make_in_maps(q, k, v, Wq, bq, Wk, bk, Wv, bv, Wo, bo):
    q, k, v = (np.asarray(x, np.float32) for x in (q, k, v))
    Wq, Wk, Wv, Wo = (np.asarray(x, np.float32) for x in (Wq, Wk, Wv, Wo))
    bq, bk, bv, bo = (np.asarray(x, np.float32) for x in (bq, bk, bv, bo))
    cc, ss = _rot_patterns()
    ones1 = np.ones((1, 128), np.float32)
    in_maps = []
    for c in range(NCORE):
        b, g = divmod(c, G)
        gs = slice(g * DL, (g + 1) * DL)
        in_maps.append({
            "qT": np.ascontiguousarray(q[b].T).astype(bf16),
            "kT": np.ascontiguousarray(k[b].T).astype(bf16),
            "vT": np.ascontiguousarray(v[b].T).astype(bf16),
            "wqT": np.ascontiguousarray(Wq[gs, :].T).astype(bf16),
            "wkT": np.ascontiguousarray(Wk[gs, :].T).astype(bf16),
            "wvT": np.ascontiguousarray(Wv[gs, :].T).astype(bf16),
            "woT": np.ascontiguousarray(Wo[:, gs].T).astype(bf16),
            "bqp": np.ascontiguousarray(bq[gs].reshape(NPAIR, 128).T),
            "bkp": np.ascontiguousarray(bk[gs].reshape(NPAIR, 128).T),
            "bv": np.ascontiguousarray(bv[gs][None, :]).astype(bf16),
            "ones1": ones1.astype(bf16),
            "cc": cc.astype(bf16), "ss": ss.astype(bf16),
        })
    return in_maps


def run(inputs: dict, trace: bool = False, tmpdir: str | None = None):
    """Returns (out [B, QL, DIM] f32, exec_time_ns or None)."""
    from concourse.bass_utils import run_bass_kernel_spmd

    nc = _build_nc()
    in_maps = _make_in_maps(**inputs)
    res = run_bass_kernel_spmd(nc, in_maps, list(range(NCORE)), trace=trace,
                               tmpdir=tmpdir)
    bo = np.asarray(inputs["bo"], np.float32)
    outs = [res.results[i]["out"] for i in range(NCORE)]
    out = np.stack([outs[G * b] + outs[G * b + 1] for b in range(B)])
    out += bo[None, None, :]
    return out.astype(np.float32), res.exec_time_ns


def kernel(**inputs) -> np.ndarray:
    out, _ = run(inputs, trace=False)
    return out


# revision 25
# speedup vs baseline: 1.5233x; 1.5233x over previous
"""Trainium2 Bass kernel for nn_MultiHeadCrossAttention (v2).

Sharding: 8 cores = 4 batches x 2 head-groups (8 heads each).

v2 over baseline: softmax exp is split between ACT (table exp) and DVE
(bit-trick fast-exp producing bf16 bit patterns via int16 tensor_scalar),
attention av-matmuls are pipelined *within* each (pair, c2) unit at lag-2
behind the score matmuls (tiny et footprint, no PE stall on exp drain),
qkproj bias-adds run on ACT (Identity+bias), normalize multiplies run on
GPSIMD, and staging DMAs are ordered weights-first.
"""

import sys

sys.path.insert(0, "/opt/trn_rl_repo")

import numpy as np
import ml_dtypes
from contextlib import ExitStack

import concourse.bass as bass
import concourse.bacc as bacc
import concourse.mybir as mybir
from concourse.tile import TileContext

DIM = 1024
H = 16
HD = 64
ROT = 32
B = 4
QL = 2048
KL = 2048
G = 2                # head-group (tensor-parallel) factor
HL = H // G          # 8 local heads
DL = HL * HD         # 512 local feature dims
NPAIR = HL // 2      # 4 head pairs
NCORE = 8

F32 = mybir.dt.float32
BF16 = mybir.dt.bfloat16
I16 = mybir.dt.int16
AFT = mybir.ActivationFunctionType
ALU = mybir.AluOpType
bf16 = ml_dtypes.bfloat16

# fast-exp: bf16 bits of exp(0.125*s) ~= round(s*FE_A + FE_B) as int16
FE_A = float(0.125 * np.log2(np.e) * 128.0)
FE_B = 16256.0

# exp engine split by half-tile index (j*4 + h*2 + n) % 16: 6/16 on DVE
DVE_HALF = {1, 4, 7, 10, 12, 15}

_NC_CACHE = {}


def _rot_patterns():
    """cc/ss blend patterns [128, QL] for the stream-shuffle rotary."""
    inv_freq = 1.0 / (10000.0 ** (np.arange(0, ROT, 2, dtype=np.float64) / ROT))
    t = np.arange(QL, dtype=np.float64)
    freqs = t[:, None] * inv_freq[None, :]          # [QL, 16]
    cos_p = np.ones((HD, QL), np.float64)
    sin_p = np.zeros((HD, QL), np.float64)
    for d in range(ROT):
        j = d // 2
        cos_p[d] = np.cos(freqs[:, j])
        sin_p[d] = np.sin(freqs[:, j]) * (-1.0 if d % 2 == 0 else 1.0)
    cc = np.tile(cos_p, (2, 1)).astype(np.float32)  # [128, QL]
    ss = np.tile(sin_p, (2, 1)).astype(np.float32)
    return cc, ss


def _build_nc():
    if "nc" in _NC_CACHE:
        return _NC_CACHE["nc"]
    nc = bacc.Bacc("TRN2", target_bir_lowering=False)

    d = {}
    for name, shape, dt in [
        ("qT", [DIM, QL], BF16), ("kT", [DIM, KL], BF16), ("vT", [DIM, KL], BF16),
        ("wqT", [DIM, DL], BF16), ("wkT", [DIM, DL], BF16), ("wvT", [DIM, DL], BF16),
        ("woT", [DL, DIM], BF16),
        ("bqp", [128, NPAIR], F32), ("bkp", [128, NPAIR], F32),
        ("bv", [1, DL], BF16), ("ones1", [1, 128], BF16),
        ("cc", [128, QL], BF16), ("ss", [128, QL], BF16),
    ]:
        d[name] = nc.dram_tensor(name, shape, dt, kind="ExternalInput")
    out_d = nc.dram_tensor("out", [QL, DIM], F32, kind="ExternalOutput")

    qT_t = d["qT"].rearrange("(a p) n -> a p n", p=128)     # [8, 128, QL]
    kT_t = d["kT"].rearrange("(a p) n -> a p n", p=128)
    vT_t = d["vT"].rearrange("(a p) n -> a p n", p=128)
    wqT_t = d["wqT"].rearrange("(a p) n -> a p n", p=128)   # [8, 128, DL]
    wkT_t = d["wkT"].rearrange("(a p) n -> a p n", p=128)
    wvT_t = d["wvT"].rearrange("(a p) n -> a p n", p=128)
    woT_t = d["woT"].rearrange("(a p) n -> a p n", p=128)   # [4, 128, DIM]
    out_t = out_d.rearrange("(a p) n -> a p n", p=128)      # [16, 128, DIM]

    SWAP_MASK = [(j + 1 if j % 2 == 0 else j - 1) for j in range(32)]

    with TileContext(nc) as tc, ExitStack() as top:
        # ---------------- constants & weights (DMA'd first) ----------------
        def alt(a):
            return nc.sync if a % 2 == 0 else nc.gpsimd

        # DMA issue order = consumption order: wk+bk -> kT (halves, both
        # queues) -> rotary consts -> v weights+data -> q weights+data -> wo.
        consts = top.enter_context(tc.tile_pool(name="consts", bufs=1))
        bk_s = consts.tile([128, NPAIR], F32)
        nc.sync.dma_start(out=bk_s, in_=d["bkp"][:, :])
        bq_s = consts.tile([128, NPAIR], F32)
        nc.gpsimd.dma_start(out=bq_s, in_=d["bqp"][:, :])
        bv_s = consts.tile([1, DL], BF16)
        nc.gpsimd.dma_start(out=bv_s, in_=d["bv"][:, :])
        ones_s = consts.tile([1, 128], BF16)
        nc.gpsimd.dma_start(out=ones_s, in_=d["ones1"][:, :])

        wpool = top.enter_context(tc.tile_pool(name="wpool", bufs=1))
        wks = [wpool.tile([128, DL], BF16, name=f"wks{i}") for i in range(8)]
        for a in range(8):
            alt(a).dma_start(out=wks[a], in_=wkT_t[a])

        etp = top.enter_context(tc.tile_pool(name="etp", bufs=16))
        stage = top.enter_context(tc.tile_pool(name="stage", bufs=8))
        ks = [stage.tile([128, KL], BF16, tag="stage", name=f"ks{i}") for i in range(8)]
        for half in range(2):
            for a in range(8):
                alt(a).dma_start(out=ks[a][:, half * 1024:(half + 1) * 1024],
                                 in_=kT_t[a][:, half * 1024:(half + 1) * 1024])

        cc_s = consts.tile([128, QL], BF16)
        nc.gpsimd.dma_start(out=cc_s, in_=d["cc"][:, :])
        ss_s = consts.tile([128, QL], BF16)
        nc.sync.dma_start(out=ss_s, in_=d["ss"][:, :])

        wvs = [wpool.tile([128, DL], BF16, name=f"wvs{i}") for i in range(8)]
        for a in range(8):
            alt(a).dma_start(out=wvs[a], in_=wvT_t[a])
        ve = [etp.tile([128, 1024], BF16, tag="et", name=f"ve{i}") for i in range(8)]
        for a in range(8):
            alt(a).dma_start(out=ve[a], in_=vT_t[a][:, 0:1024])
        wqs = [wpool.tile([128, DL], BF16, name=f"wqs{i}") for i in range(8)]
        for a in range(8):
            alt(a).dma_start(out=wqs[a], in_=wqT_t[a])
        qe = [etp.tile([128, 1024], BF16, tag="et", name=f"qe{i}") for i in range(8)]
        for a in range(8):
            alt(a).dma_start(out=qe[a], in_=qT_t[a][:, 0:1024])
        vs = [stage.tile([128, 1024], BF16, tag="stage", name=f"vs{i}") for i in range(8)]
        for a in range(8):
            alt(a).dma_start(out=vs[a], in_=vT_t[a][:, 1024:2048])
        qs = [stage.tile([128, QL], BF16, tag="stage", name=f"qs{i}") for i in range(8)]
        for a in range(8):
            alt(a).dma_start(out=qs[a], in_=qT_t[a])
        wo_s = [wpool.tile([128, DIM], BF16, name=f"wo{i}") for i in range(NPAIR)]
        for i in range(NPAIR):
            alt(i).dma_start(out=wo_s[i], in_=woT_t[i])

        # Warm the ACT exp table early.
        warm = consts.tile([1, 8], F32)
        nc.scalar.activation(out=warm, in_=ones_s[0:1, 0:8], func=AFT.Exp)

        # ---------------- persistent activations ----------------
        qh_pool = top.enter_context(tc.tile_pool(name="qh", bufs=1))
        qhT = [qh_pool.tile([128, QL], BF16, name=f"qh{i}") for i in range(NPAIR)]
        kh_pool = top.enter_context(tc.tile_pool(name="kh", bufs=1))
        khT = [kh_pool.tile([128, KL], BF16, name=f"kh{i}") for i in range(NPAIR)]
        vh_pool = top.enter_context(tc.tile_pool(name="vh", bufs=1))
        # Per kl-tile: 4 pairs x [vh_even(64) | 1 | 1 | vh_odd(64)] bf16.
        vh = [vh_pool.tile([128, NPAIR * 130], BF16, name=f"vh{i}") for i in range(16)]
        at_pool = top.enter_context(tc.tile_pool(name="atn", bufs=1))
        apT = [at_pool.tile([128, QL], BF16, name=f"apT{i}") for i in range(NPAIR)]

        # ---------------- working pools ----------------
        rtmp = top.enter_context(tc.tile_pool(name="rtmp", bufs=1))
        aup = top.enter_context(tc.tile_pool(name="aup", bufs=4))
        btp = top.enter_context(tc.tile_pool(name="btp", bufs=4))
        rcp = top.enter_context(tc.tile_pool(name="rcp", bufs=6))
        outst = top.enter_context(tc.tile_pool(name="outst", bufs=3))
        dscr = top.enter_context(tc.tile_pool(name="dscr", bufs=8, space="DRAM"))
        psS = top.enter_context(tc.tile_pool(name="psS", bufs=4, space="PSUM"))
        psP = top.enter_context(tc.tile_pool(name="psP", bufs=4, space="PSUM"))

        # ---------------- helpers ----------------
        def rotary_half(dst, mt, c2):
            cs = slice(c2 * 1024, (c2 + 1) * 1024)
            sw = rtmp.tile([128, 1024], BF16, tag="sw")
            nc.vector.stream_shuffle(out=sw, in_=dst[mt][:, cs], mask=SWAP_MASK)
            t1 = rtmp.tile([128, 1024], BF16, tag="t1")
            nc.vector.tensor_tensor(out=t1, in0=sw, in1=ss_s[:, cs], op=ALU.mult)
            t2 = rtmp.tile([128, 1024], BF16, tag="t2")
            nc.vector.tensor_tensor(out=t2, in0=dst[mt][:, cs], in1=cc_s[:, cs], op=ALU.mult)
            nc.vector.tensor_tensor(out=dst[mt][:, cs], in0=t1, in1=t2, op=ALU.add)

        def rotary(dst, mt):
            for c2 in range(2):
                rotary_half(dst, mt, c2)

        def k_rhs(a, c2, n):
            return ks[a][:, c2 * 1024 + n * 512:c2 * 1024 + (n + 1) * 512]

        def q0_rhs(a, c2, n):
            # c2=0 half lives in the early tiles, c2=1 in the stage ring
            if c2 == 0:
                return qe[a][:, n * 512:(n + 1) * 512]
            return qs[a][:, 1024 + n * 512:1024 + (n + 1) * 512]

        def q_rhs(a, c2, n):
            return qs[a][:, c2 * 1024 + n * 512:c2 * 1024 + (n + 1) * 512]

        def qkproj_chunk(rhs_fn, ws, b_s, dst, mt, c2, n, pool, ptag):
            """One [128, 512] chunk of a q/k projection + ACT bias-add."""
            ps = pool.tile([128, 512], F32, tag=ptag, name=f"pj{mt}{c2}{n}")
            for a in range(8):
                nc.tensor.matmul(
                    ps,
                    lhsT=ws[a][:, mt * 128:(mt + 1) * 128],
                    rhs=rhs_fn(a, c2, n),
                    start=(a == 0), stop=(a == 7),
                )
            nc.scalar.activation(
                out=dst[mt][:, c2 * 1024 + n * 512:c2 * 1024 + (n + 1) * 512],
                in_=ps, func=AFT.Identity, bias=b_s[:, mt:mt + 1])

        def vproj_tile(t):
            ps = psS.tile([128, DL], F32, tag="S", name=f"vp{t}")
            for a in range(8):
                lhs = (ve[a][:, t * 128:(t + 1) * 128] if t < 8
                       else vs[a][:, (t - 8) * 128:(t - 7) * 128])
                nc.tensor.matmul(ps, lhsT=lhs, rhs=wvs[a],
                                 start=(a == 0), stop=False)
            nc.tensor.matmul(ps, lhsT=ones_s, rhs=bv_s, start=False, stop=True)
            vtr = vh[t].rearrange("p (g h e) -> p g h e", h=2, e=65)
            nc.gpsimd.memset(vtr[:, :, :, 64:65], 1.0)
            psr = ps.rearrange("p (g h e) -> p g h e", h=2, e=64)
            nc.scalar.activation(out=vtr[:, :, :, 0:64], in_=psr, func=AFT.Copy)

        def outproj_chunk(qt, dc):
            ps = psS.tile([128, 512], F32, tag="S", name=f"op{qt}{dc}")
            for pi in range(NPAIR):
                nc.tensor.matmul(
                    ps,
                    lhsT=apT[pi][:, qt * 128:(qt + 1) * 128],
                    rhs=wo_s[pi][:, dc * 512:(dc + 1) * 512],
                    start=(pi == 0), stop=(pi == NPAIR - 1),
                )
            ot = outst.tile([128, 512], F32, tag="o")
            if dc == 0:
                nc.vector.tensor_copy(out=ot, in_=ps)
            else:
                nc.scalar.activation(out=ot, in_=ps, func=AFT.Copy)
            eng = nc.sync if qt % 2 == 0 else nc.gpsimd
            eng.dma_start(out=out_t[qt][:, dc * 512:(dc + 1) * 512], in_=ot)

        # ---------------- phase A ----------------
        # kproj (c2-outer so compute paces with the half-split kT DMAs),
        # the ve-sourced half of vproj, and the c2=0 half of qproj0. The
        # vs-sourced vproj half and qproj0-c2=1 run as unit-0 fillers.
        for c2 in range(2):
            for n in range(2):
                for mt in range(NPAIR):
                    qkproj_chunk(k_rhs, wks, bk_s, khT, mt, c2, n, psP, "P")
                    if c2 == 1 and n == 1:
                        rotary(khT, mt)
        for t in range(8):
            vproj_tile(t)
        for n in range(2):
            qkproj_chunk(q0_rhs, wqs, bq_s, qhT, 0, 0, n, psP, "P")
        rotary_half(qhT, 0, 0)

        # ---------------- phase B: 8 units, continuous attn pipeline ----
        LAG = 2
        ustate = {}   # ug -> dict(p, c2, ets, accs)

        def attn_advance(g):
            """Emit the attn accumulation step for global position g, plus the
            unit drain (denominators, normalize) when its last step lands."""
            if g < 0 or g >= 128:
                return
            ug, tg = divmod(g, 16)
            st = ustate[ug]
            p, c2 = st["p"], st["c2"]
            if tg == 0:
                st["accs"] = {
                    (h, n): psP.tile([128, 512], F32, tag="P",
                                     name=f"pa{ug}{h}{n}")
                    for h in range(2) for n in range(2)}
            accs = st["accs"]
            for h in range(2):
                lhs = vh[tg][:, p * 130 + h * 65: p * 130 + (h + 1) * 65]
                for n in range(2):
                    nc.tensor.matmul(
                        accs[(h, n)][0:65, :],
                        lhsT=lhs,
                        rhs=st["ets"][(tg, h)][:, n * 512:(n + 1) * 512],
                        start=(tg == 0), stop=(tg == 15),
                    )
            if tg < 15:
                return
            # drain: copy out, reciprocal of the denominator row in place,
            # partition-broadcast it, normalize on gpsimd
            for h in range(2):
                for n in range(2):
                    atu = aup.tile([128, 512], F32, tag="atu")
                    if (h + n) % 2 == 0:
                        nc.vector.tensor_copy(out=atu[0:65, :],
                                              in_=accs[(h, n)][0:65, :])
                    else:
                        nc.scalar.activation(out=atu[0:65, :],
                                             in_=accs[(h, n)][0:65, :],
                                             func=AFT.Copy)
                    ds = dscr.tile([1, 512], F32, tag="dsc")
                    nc.sync.dma_start(out=ds, in_=atu[64:65, :])
                    rc = rcp.tile([128, 4], F32, tag="rc")
                    nc.sync.dma_start(out=rc, in_=ds.rearrange("a (p e) -> (a p) e", p=128))
                    nc.vector.reciprocal(out=rc, in_=rc)
                    ds2 = dscr.tile([1, 512], F32, tag="ds2")
                    nc.sync.dma_start(out=ds2.rearrange("a (p e) -> (a p) e", p=128), in_=rc)
                    bt = btp.tile([64, 512], F32, tag="bc")
                    nc.sync.dma_start(out=bt, in_=ds2[0:1, :].to_broadcast([64, 512]))
                    nc.gpsimd.tensor_tensor(
                        out=apT[p][h * 64:(h + 1) * 64,
                                   c2 * 1024 + n * 512:c2 * 1024 + (n + 1) * 512],
                        in0=atu[0:64, :], in1=bt, op=ALU.mult)
            del ustate[ug]

        def emit_unit(u):
            p, c2 = divmod(u, 2)
            ets = {}
            ustate[u] = {"p": p, "c2": c2, "ets": ets, "accs": None}

            for j in range(16):
                # h-alternating emission: adjacent score MMs use disjoint
                # row groups so the PE can co-execute them
                etj = {}
                for h in range(2):
                    etj[h] = etp.tile([128, 1024], BF16, tag="et",
                                      name=f"et{u}{j}{h}")
                    ets[(j, h)] = etj[h]
                for n in range(2):
                    for h in range(2):
                        ps = psS.tile([128, 512], F32, tag="S", name=f"s{u}{j}{h}{n}")
                        nc.tensor.matmul(
                            ps,
                            lhsT=khT[p][h * 64:(h + 1) * 64,
                                        j * 128:(j + 1) * 128],
                            rhs=qhT[p][h * 64:(h + 1) * 64,
                                       c2 * 1024 + n * 512:
                                       c2 * 1024 + (n + 1) * 512],
                            start=True, stop=True,
                            tile_position=(h * 64, 0),
                        )
                        if (j * 4 + h * 2 + n) % 16 in DVE_HALF:
                            nc.vector.tensor_scalar(
                                out=etj[h].bitcast(I16)[:, n * 512:(n + 1) * 512],
                                in0=ps, scalar1=FE_A, scalar2=FE_B,
                                op0=ALU.mult, op1=ALU.add)
                        else:
                            nc.scalar.activation(
                                out=etj[h][:, n * 512:(n + 1) * 512], in_=ps,
                                func=AFT.Exp, scale=0.125)
                attn_advance(16 * u + j - LAG)
                if u == 7 and j >= 5:
                    # unit 6's apT c2=0 normalize lands ~iter 1-4; 11 chunks
                    # here, the remaining 21 in the tail
                    ci = j - 5
                    outproj_chunk(ci // 2, ci % 2)
                # unit-0 fillers: the vs-sourced vproj half + qproj0's c2=1
                if u == 0:
                    if j < 4:
                        vproj_tile(8 + 2 * j)
                        vproj_tile(9 + 2 * j)
                    elif j in (9, 11):
                        qkproj_chunk(q0_rhs, wqs, bq_s, qhT, 0, 1, (j - 9) // 2,
                                     psS, "S")
                    elif j == 13:
                        rotary_half(qhT, 0, 1)
                # filler: qproj pair 1 in unit 1, pair 2 in units 2-3,
                # pair 3 in units 4-5 (1 chunk per 4 or 8 iters); rotary at
                # j=15 so it overlaps the unit edge.
                if u == 1 and j % 4 == 1:
                    c2f, nf = divmod(j // 4, 2)
                    qkproj_chunk(q_rhs, wqs, bq_s, qhT, 1, c2f, nf, psS, "S")
                elif u in (2, 3, 4, 5) and j % 8 == 1:
                    idx = (u - 2) * 2 + j // 8
                    pair = 2 + idx // 4
                    c2f, nf = divmod(idx % 4, 2)
                    qkproj_chunk(q_rhs, wqs, bq_s, qhT, pair, c2f, nf, psS, "S")
                if j == 15 and u in (1, 3, 5):
                    rotary(qhT, (u + 1) // 2)

        for u in range(8):
            emit_unit(u)
        for g in range(128 - LAG, 128):
            attn_advance(g)

        # ---------------- tail: remaining out-projection ----------------
        for ci in range(11, 32):
            outproj_chunk(ci // 2, ci % 2)

    nc.compile()
    _NC_CACHE["nc"] = nc
    return nc


def _make_in_maps(q, k, v, Wq, bq, Wk, bk, Wv, bv, Wo, bo):
    q, k, v = (np.asarray(x, np.float32) for x in (q, k, v))
    Wq, Wk, Wv, Wo = (np.asarray(x, np.float32) for x in (Wq, Wk, Wv, Wo))
    bq, bk, bv, bo = (np.asarray(x, np.float32) for x in (bq, bk, bv, bo))
    cc, ss = _rot_patterns()
    ones1 = np.ones((1, 128), np.float32)
    in_maps = []
    for c in range(NCORE):
        b, g = divmod(c, G)
        gs = slice(g * DL, (g + 1) * DL)
        in_maps.append({
            "qT": np.ascontiguousarray(q[b].T).astype(bf16),
            "kT": np.ascontiguousarray(k[b].T).astype(bf16),
            "vT": np.ascontiguousarray(v[b].T).astype(bf16),
            "wqT": np.ascontiguousarray(Wq[gs, :].T).astype(bf16),
            "wkT": np.ascontiguousarray(Wk[gs, :].T).astype(bf16),
            "wvT": np.ascontiguousarray(Wv[gs, :].T).astype(bf16),
            "woT": np.ascontiguousarray(Wo[:, gs].T).astype(bf16),
            "bqp": np.ascontiguousarray(bq[gs].reshape(NPAIR, 128).T),
            "bkp": np.ascontiguousarray(bk[gs].reshape(NPAIR, 128).T),
            "bv": np.ascontiguousarray(bv[gs][None, :]).astype(bf16),
            "ones1": ones1.astype(bf16),
            "cc": cc.astype(bf16), "ss": ss.astype(bf16),
        })
    return in_maps


def run(inputs: dict, trace: bool = False, tmpdir: str | None = None):
    """Returns (out [B, QL, DIM] f32, exec_time_ns or None)."""
    from concourse.bass_utils import run_bass_kernel_spmd

    nc = _build_nc()
    in_maps = _make_in_maps(**inputs)
    res = run_bass_kernel_spmd(nc, in_maps, list(range(NCORE)), trace=trace,
                               tmpdir=tmpdir)
    bo = np.asarray(inputs["bo"], np.float32)
    outs = [res.results[i]["out"] for i in range(NCORE)]
    out = np.stack([outs[G * b] + outs[G * b + 1] for b in range(B)])
    out += bo[None, None, :]
    return out.astype(np.float32), res.exec_time_ns


def kernel(**inputs) -> np.ndarray:
    out, _ = run(inputs, trace=False)
    return out


# revision 35
# speedup vs baseline: 1.6416x; 1.0777x over previous
"""Trainium2 Bass kernel for nn_MultiHeadCrossAttention (v2).

Sharding: 8 cores = 4 batches x 2 head-groups (8 heads each).

v2 over baseline: softmax exp is split between ACT (table exp) and DVE
(bit-trick fast-exp producing bf16 bit patterns via int16 tensor_scalar),
attention av-matmuls are pipelined *within* each (pair, c2) unit at lag-2
behind the score matmuls (tiny et footprint, no PE stall on exp drain),
qkproj bias-adds run on ACT (Identity+bias), normalize multiplies run on
GPSIMD, and staging DMAs are ordered weights-first.
"""

import sys

sys.path.insert(0, "/opt/trn_rl_repo")

import numpy as np
import ml_dtypes
from contextlib import ExitStack

import concourse.bass as bass
import concourse.bacc as bacc
import concourse.mybir as mybir
from concourse.tile import TileContext

DIM = 1024
H = 16
HD = 64
ROT = 32
B = 4
QL = 2048
KL = 2048
G = 2                # head-group (tensor-parallel) factor
HL = H // G          # 8 local heads
DL = HL * HD         # 512 local feature dims
NPAIR = HL // 2      # 4 head pairs
NCORE = 8

F32 = mybir.dt.float32
BF16 = mybir.dt.bfloat16
I16 = mybir.dt.int16
AFT = mybir.ActivationFunctionType
ALU = mybir.AluOpType
bf16 = ml_dtypes.bfloat16

# fast-exp: bf16 bits of exp(0.125*s) ~= round(s*FE_A + FE_B) as int16
FE_A = float(0.125 * np.log2(np.e) * 128.0)
FE_B = 16256.0

# exp engine split by half-tile index (j*4 + h*2 + n) % 16: 6/16 on DVE
DVE_HALF = {1, 4, 7, 10, 12, 15}

_NC_CACHE = {}


def _rot_patterns():
    """cc/ss blend patterns [128, QL] for the stream-shuffle rotary."""
    inv_freq = 1.0 / (10000.0 ** (np.arange(0, ROT, 2, dtype=np.float64) / ROT))
    t = np.arange(QL, dtype=np.float64)
    freqs = t[:, None] * inv_freq[None, :]          # [QL, 16]
    cos_p = np.ones((HD, QL), np.float64)
    sin_p = np.zeros((HD, QL), np.float64)
    for d in range(ROT):
        j = d // 2
        cos_p[d] = np.cos(freqs[:, j])
        sin_p[d] = np.sin(freqs[:, j]) * (-1.0 if d % 2 == 0 else 1.0)
    cc = np.tile(cos_p, (2, 1)).astype(np.float32)  # [128, QL]
    ss = np.tile(sin_p, (2, 1)).astype(np.float32)
    return cc, ss


def _build_nc():
    if "nc" in _NC_CACHE:
        return _NC_CACHE["nc"]
    nc = bacc.Bacc("TRN2", target_bir_lowering=False)

    d = {}
    for name, shape, dt in [
        ("qT", [DIM, QL], BF16), ("kT", [DIM, KL], BF16), ("vT", [DIM, KL], BF16),
        ("wqT", [DIM, DL], BF16), ("wkT", [DIM, DL], BF16), ("wvT", [DIM, DL], BF16),
        ("woT", [DL, DIM], BF16),
        ("bqp", [128, NPAIR], F32), ("bkp", [128, NPAIR], F32),
        ("ones1", [1, 128], BF16),
        ("cc", [128, QL], BF16), ("ss", [128, QL], BF16),
    ]:
        d[name] = nc.dram_tensor(name, shape, dt, kind="ExternalInput")
    out_d = nc.dram_tensor("out", [QL, DIM], F32, kind="ExternalOutput")

    qT_t = d["qT"].rearrange("(a p) n -> a p n", p=128)     # [8, 128, QL]
    kT_t = d["kT"].rearrange("(a p) n -> a p n", p=128)
    vT_t = d["vT"].rearrange("(a p) n -> a p n", p=128)
    wqT_t = d["wqT"].rearrange("(a p) n -> a p n", p=128)   # [8, 128, DL]
    wkT_t = d["wkT"].rearrange("(a p) n -> a p n", p=128)
    wvT_t = d["wvT"].rearrange("(a p) n -> a p n", p=128)
    woT_t = d["woT"].rearrange("(a p) n -> a p n", p=128)   # [4, 128, DIM]
    out_t = out_d.rearrange("(a p) n -> a p n", p=128)      # [16, 128, DIM]

    SWAP_MASK = [(j + 1 if j % 2 == 0 else j - 1) for j in range(32)]

    with TileContext(nc) as tc, ExitStack() as top:
        # ---------------- constants & weights (DMA'd first) ----------------
        def alt(a):
            return nc.sync if a % 2 == 0 else nc.gpsimd

        # DMA issue order = consumption order: wk+bk -> kT (halves, both
        # queues) -> rotary consts -> v weights+data -> q weights+data -> wo.
        consts = top.enter_context(tc.tile_pool(name="consts", bufs=1))
        bk_s = consts.tile([128, NPAIR], F32)
        nc.sync.dma_start(out=bk_s, in_=d["bkp"][:, :])
        bq_s = consts.tile([128, NPAIR], F32)
        nc.gpsimd.dma_start(out=bq_s, in_=d["bqp"][:, :])
        ones_s = consts.tile([1, 128], BF16)
        nc.gpsimd.dma_start(out=ones_s, in_=d["ones1"][:, :])

        wpool = top.enter_context(tc.tile_pool(name="wpool", bufs=1))
        wks = [wpool.tile([128, DL], BF16, name=f"wks{i}") for i in range(8)]
        for a in range(8):
            alt(a).dma_start(out=wks[a], in_=wkT_t[a])

        etp = top.enter_context(tc.tile_pool(name="etp", bufs=16))
        stage = top.enter_context(tc.tile_pool(name="stage", bufs=8))
        ks = [stage.tile([128, KL], BF16, tag="stage", name=f"ks{i}") for i in range(8)]
        for half in range(2):
            for a in range(8):
                alt(a).dma_start(out=ks[a][:, half * 1024:(half + 1) * 1024],
                                 in_=kT_t[a][:, half * 1024:(half + 1) * 1024])

        cc_s = consts.tile([128, QL], BF16)
        nc.gpsimd.dma_start(out=cc_s, in_=d["cc"][:, :])
        ss_s = consts.tile([128, QL], BF16)
        nc.sync.dma_start(out=ss_s, in_=d["ss"][:, :])

        wqs = [wpool.tile([128, DL], BF16, name=f"wqs{i}") for i in range(8)]
        for a in range(8):
            alt(a).dma_start(out=wqs[a], in_=wqT_t[a])
        qe = [etp.tile([128, 1024], BF16, tag="et", name=f"qe{i}") for i in range(8)]
        for a in range(8):
            alt(a).dma_start(out=qe[a], in_=qT_t[a][:, 0:1024])
        wvs = [wpool.tile([128, DL], BF16, name=f"wvs{i}") for i in range(8)]
        for a in range(8):
            alt(a).dma_start(out=wvs[a], in_=wvT_t[a])
        ve = [etp.tile([128, 1024], BF16, tag="et", name=f"ve{i}") for i in range(8)]
        for a in range(8):
            alt(a).dma_start(out=ve[a], in_=vT_t[a][:, 0:1024])
        vs = [stage.tile([128, 1024], BF16, tag="stage", name=f"vs{i}") for i in range(8)]
        for a in range(8):
            alt(a).dma_start(out=vs[a], in_=vT_t[a][:, 1024:2048])
        qs = [stage.tile([128, QL], BF16, tag="stage", name=f"qs{i}") for i in range(8)]
        for a in range(8):
            alt(a).dma_start(out=qs[a], in_=qT_t[a])
        wo_s = [wpool.tile([128, DIM], BF16, name=f"wo{i}") for i in range(NPAIR)]
        for i in range(NPAIR):
            alt(i).dma_start(out=wo_s[i], in_=woT_t[i])

        # Warm the ACT exp table early.
        warm = consts.tile([1, 8], F32)
        nc.scalar.activation(out=warm, in_=ones_s[0:1, 0:8], func=AFT.Exp)

        # ---------------- persistent activations ----------------
        qh_pool = top.enter_context(tc.tile_pool(name="qh", bufs=1))
        qhT = [qh_pool.tile([128, QL], BF16, name=f"qh{i}") for i in range(NPAIR)]
        kh_pool = top.enter_context(tc.tile_pool(name="kh", bufs=1))
        khT = [kh_pool.tile([128, KL], BF16, name=f"kh{i}") for i in range(NPAIR)]
        vh_pool = top.enter_context(tc.tile_pool(name="vh", bufs=1))
        # Per kl-tile: 4 pairs x [vh_even(64) | 1 | 1 | vh_odd(64)] bf16.
        vh = [vh_pool.tile([128, NPAIR * 130], BF16, name=f"vh{i}") for i in range(16)]
        at_pool = top.enter_context(tc.tile_pool(name="atn", bufs=1))
        apT = [at_pool.tile([128, QL], BF16, name=f"apT{i}") for i in range(NPAIR)]

        # ---------------- working pools ----------------
        rtmp = top.enter_context(tc.tile_pool(name="rtmp", bufs=1))
        aup = top.enter_context(tc.tile_pool(name="aup", bufs=4))
        btp = top.enter_context(tc.tile_pool(name="btp", bufs=4))
        rcp = top.enter_context(tc.tile_pool(name="rcp", bufs=6))
        outst = top.enter_context(tc.tile_pool(name="outst", bufs=3))
        dscr = top.enter_context(tc.tile_pool(name="dscr", bufs=8, space="DRAM"))
        psS = top.enter_context(tc.tile_pool(name="psS", bufs=4, space="PSUM"))
        psP = top.enter_context(tc.tile_pool(name="psP", bufs=4, space="PSUM"))

        # ---------------- helpers ----------------
        def rotary_half(dst, mt, c2):
            cs = slice(c2 * 1024, (c2 + 1) * 1024)
            sw = rtmp.tile([128, 1024], BF16, tag="sw")
            nc.vector.stream_shuffle(out=sw, in_=dst[mt][:, cs], mask=SWAP_MASK)
            t1 = rtmp.tile([128, 1024], BF16, tag="t1")
            nc.vector.tensor_tensor(out=t1, in0=sw, in1=ss_s[:, cs], op=ALU.mult)
            t2 = rtmp.tile([128, 1024], BF16, tag="t2")
            nc.vector.tensor_tensor(out=t2, in0=dst[mt][:, cs], in1=cc_s[:, cs], op=ALU.mult)
            nc.vector.tensor_tensor(out=dst[mt][:, cs], in0=t1, in1=t2, op=ALU.add)

        def rotary(dst, mt):
            for c2 in range(2):
                rotary_half(dst, mt, c2)

        def k_rhs(a, c2, n):
            return ks[a][:, c2 * 1024 + n * 512:c2 * 1024 + (n + 1) * 512]

        def q0_rhs(a, c2, n):
            # c2=0 half lives in the early tiles, c2=1 in the stage ring
            if c2 == 0:
                return qe[a][:, n * 512:(n + 1) * 512]
            return qs[a][:, 1024 + n * 512:1024 + (n + 1) * 512]

        def q_rhs(a, c2, n):
            return qs[a][:, c2 * 1024 + n * 512:c2 * 1024 + (n + 1) * 512]

        def qkproj_chunk(rhs_fn, ws, b_s, dst, mt, c2, n, pool, ptag):
            """One [128, 512] chunk of a q/k projection + ACT bias-add."""
            ps = pool.tile([128, 512], F32, tag=ptag, name=f"pj{mt}{c2}{n}")
            for a in range(8):
                nc.tensor.matmul(
                    ps,
                    lhsT=ws[a][:, mt * 128:(mt + 1) * 128],
                    rhs=rhs_fn(a, c2, n),
                    start=(a == 0), stop=(a == 7),
                )
            nc.scalar.activation(
                out=dst[mt][:, c2 * 1024 + n * 512:c2 * 1024 + (n + 1) * 512],
                in_=ps, func=AFT.Identity, bias=b_s[:, mt:mt + 1])

        def vproj_tile(t):
            # bv is folded into bo host-side (bo += Wo @ bv), so no bias MM
            ps = psS.tile([128, DL], F32, tag="S", name=f"vp{t}")
            for a in range(8):
                lhs = (ve[a][:, t * 128:(t + 1) * 128] if t < 8
                       else vs[a][:, (t - 8) * 128:(t - 7) * 128])
                nc.tensor.matmul(ps, lhsT=lhs, rhs=wvs[a],
                                 start=(a == 0), stop=(a == 7))
            vtr = vh[t].rearrange("p (g h e) -> p g h e", h=2, e=65)
            nc.gpsimd.memset(vtr[:, :, :, 64:65], 1.0)
            psr = ps.rearrange("p (g h e) -> p g h e", h=2, e=64)
            nc.scalar.activation(out=vtr[:, :, :, 0:64], in_=psr, func=AFT.Copy)

        def outproj_chunk(qt, dc):
            ps = psS.tile([128, 512], F32, tag="S", name=f"op{qt}{dc}")
            for pi in range(NPAIR):
                nc.tensor.matmul(
                    ps,
                    lhsT=apT[pi][:, qt * 128:(qt + 1) * 128],
                    rhs=wo_s[pi][:, dc * 512:(dc + 1) * 512],
                    start=(pi == 0), stop=(pi == NPAIR - 1),
                )
            ot = outst.tile([128, 512], F32, tag="o")
            if dc == 0:
                nc.vector.tensor_copy(out=ot, in_=ps)
            else:
                nc.scalar.activation(out=ot, in_=ps, func=AFT.Copy)
            eng = nc.sync if qt % 2 == 0 else nc.gpsimd
            eng.dma_start(out=out_t[qt][:, dc * 512:(dc + 1) * 512], in_=ot)

        # ---------------- phase A ----------------
        # kproj (c2-outer so compute paces with the half-split kT DMAs),
        # the ve-sourced half of vproj, and the c2=0 half of qproj0. The
        # vs-sourced vproj half and qproj0-c2=1 run as unit-0 fillers.
        for c2 in range(2):
            for n in range(2):
                for mt in range(NPAIR):
                    qkproj_chunk(k_rhs, wks, bk_s, khT, mt, c2, n, psP, "P")
                    if c2 == 1 and n == 1:
                        rotary(khT, mt)
        for n in range(2):
            qkproj_chunk(q0_rhs, wqs, bq_s, qhT, 0, 0, n, psP, "P")
        rotary_half(qhT, 0, 0)
        for t in range(8):
            vproj_tile(t)

        # ---------------- phase B: 8 units, continuous attn pipeline ----
        LAG = 2
        ustate = {}   # ug -> dict(p, c2, ets, accs)

        def attn_advance(g, ah):
            """Emit head-pair ah's attn accumulation MMs for global position
            g; on the last step of a unit, emit the whole unit's drain."""
            if g < 0 or g >= 128:
                return
            ug, tg = divmod(g, 16)
            st = ustate[ug]
            p, c2 = st["p"], st["c2"]
            if tg == 0 and ah == 0:
                st["accs"] = {
                    (h, n): psP.tile([128, 512], F32, tag="P",
                                     name=f"pa{ug}{h}{n}")
                    for h in range(2) for n in range(2)}
            accs = st["accs"]
            lhs = vh[tg][:, p * 130 + ah * 65: p * 130 + (ah + 1) * 65]
            for n in range(2):
                nc.tensor.matmul(
                    accs[(ah, n)][0:65, :],
                    lhsT=lhs,
                    rhs=st["ets"][(tg, ah)][:, n * 512:(n + 1) * 512],
                    start=(tg == 0), stop=(tg == 15),
                )
            if tg < 15 or ah < 1:
                return
            # unit drain: copies out, batched denominator reciprocal
            # round-trip, per-chunk broadcast + normalize
            atus = {}
            ds4 = dscr.tile([1, 2048], F32, tag="dsc")
            for h in range(2):
                for n in range(2):
                    atu = aup.tile([128, 512], F32, tag="atu")
                    atus[(h, n)] = atu
                    if (h + n) % 2 == 0:
                        nc.vector.tensor_copy(out=atu[0:65, :],
                                              in_=accs[(h, n)][0:65, :])
                    else:
                        nc.scalar.activation(out=atu[0:65, :],
                                             in_=accs[(h, n)][0:65, :],
                                             func=AFT.Copy)
                    k = h * 2 + n
                    nc.sync.dma_start(out=ds4[:, k * 512:(k + 1) * 512],
                                      in_=atu[64:65, :])
            rc = rcp.tile([128, 16], F32, tag="rc")
            nc.sync.dma_start(out=rc, in_=ds4.rearrange("a (p e) -> (a p) e", p=128))
            nc.vector.reciprocal(out=rc, in_=rc)
            ds4b = dscr.tile([1, 2048], F32, tag="ds2")
            nc.sync.dma_start(out=ds4b.rearrange("a (p e) -> (a p) e", p=128), in_=rc)
            norm_eng = nc.vector if ug == 7 else nc.gpsimd
            for h in range(2):
                for n in range(2):
                    k = h * 2 + n
                    bt = btp.tile([64, 512], F32, tag="bc")
                    nc.sync.dma_start(
                        out=bt,
                        in_=ds4b[0:1, k * 512:(k + 1) * 512].to_broadcast([64, 512]))
                    norm_eng.tensor_tensor(
                        out=apT[p][h * 64:(h + 1) * 64,
                                   c2 * 1024 + n * 512:c2 * 1024 + (n + 1) * 512],
                        in0=atus[(h, n)][0:64, :], in1=bt, op=ALU.mult)
            del ustate[ug]

        def emit_unit(u):
            p, c2 = divmod(u, 2)
            ets = {}
            ustate[u] = {"p": p, "c2": c2, "ets": ets, "accs": None}

            for j in range(16):
                # h-alternating emission: adjacent score MMs use disjoint
                # row groups so the PE co-executes them; attn pairs fill
                # the exp-latency window between score pairs
                etj = {}
                for h in range(2):
                    etj[h] = etp.tile([128, 1024], BF16, tag="et",
                                      name=f"et{u}{j}{h}")
                    ets[(j, h)] = etj[h]
                for n in range(2):
                    for h in range(2):
                        ps = psS.tile([128, 512], F32, tag="S", name=f"s{u}{j}{h}{n}")
                        nc.tensor.matmul(
                            ps,
                            lhsT=khT[p][h * 64:(h + 1) * 64,
                                        j * 128:(j + 1) * 128],
                            rhs=qhT[p][h * 64:(h + 1) * 64,
                                       c2 * 1024 + n * 512:
                                       c2 * 1024 + (n + 1) * 512],
                            start=True, stop=True,
                            tile_position=(h * 64, 0),
                        )
                        if (j * 4 + h * 2 + n) % 16 in DVE_HALF:
                            nc.vector.tensor_scalar(
                                out=etj[h].bitcast(I16)[:, n * 512:(n + 1) * 512],
                                in0=ps, scalar1=FE_A, scalar2=FE_B,
                                op0=ALU.mult, op1=ALU.add)
                        else:
                            nc.scalar.activation(
                                out=etj[h][:, n * 512:(n + 1) * 512], in_=ps,
                                func=AFT.Exp, scale=0.125)
                    attn_advance(16 * u + j - LAG, n)
                if u == 7 and j >= 5:
                    # unit 6's apT c2=0 normalize lands ~iter 1-4; 11 chunks
                    # here, the remaining 21 in the tail
                    ci = j - 5
                    outproj_chunk(ci // 2, ci % 2)
                # unit-0 fillers: the vs-sourced vproj half + qproj0's c2=1
                if u == 0:
                    if j < 4:
                        vproj_tile(8 + 2 * j)
                        vproj_tile(9 + 2 * j)
                    elif j in (9, 11):
                        qkproj_chunk(q0_rhs, wqs, bq_s, qhT, 0, 1, (j - 9) // 2,
                                     psS, "S")
                    elif j == 13:
                        rotary_half(qhT, 0, 1)
                # filler: qproj pair 1 in unit 1, pair 2 in units 2-3,
                # pair 3 in units 4-5 (1 chunk per 4 or 8 iters); rotary at
                # j=15 so it overlaps the unit edge.
                if u == 1 and j % 4 == 1:
                    c2f, nf = divmod(j // 4, 2)
                    qkproj_chunk(q_rhs, wqs, bq_s, qhT, 1, c2f, nf, psS, "S")
                elif u in (2, 3, 4, 5) and j % 8 == 1:
                    idx = (u - 2) * 2 + j // 8
                    pair = 2 + idx // 4
                    c2f, nf = divmod(idx % 4, 2)
                    qkproj_chunk(q_rhs, wqs, bq_s, qhT, pair, c2f, nf, psS, "S")
                if j == 15 and u in (1, 3, 5):
                    rotary(qhT, (u + 1) // 2)

        for u in range(8):
            emit_unit(u)
        for g in range(128 - LAG, 128):
            attn_advance(g, 0)
            attn_advance(g, 1)

        # ---------------- tail: remaining out-projection ----------------
        for ci in range(11, 32):
            outproj_chunk(ci // 2, ci % 2)

    nc.compile()
    _NC_CACHE["nc"] = nc
    return nc


def _make_in_maps(q, k, v, Wq, bq, Wk, bk, Wv, bv, Wo, bo):
    q, k, v = (np.asarray(x, np.float32) for x in (q, k, v))
    Wq, Wk, Wv, Wo = (np.asarray(x, np.float32) for x in (Wq, Wk, Wv, Wo))
    bq, bk, bv, bo = (np.asarray(x, np.float32) for x in (bq, bk, bv, bo))
    cc, ss = _rot_patterns()
    ones1 = np.ones((1, 128), np.float32)
    in_maps = []
    for c in range(NCORE):
        b, g = divmod(c, G)
        gs = slice(g * DL, (g + 1) * DL)
        in_maps.append({
            "qT": np.ascontiguousarray(q[b].T).astype(bf16),
            "kT": np.ascontiguousarray(k[b].T).astype(bf16),
            "vT": np.ascontiguousarray(v[b].T).astype(bf16),
            "wqT": np.ascontiguousarray(Wq[gs, :].T).astype(bf16),
            "wkT": np.ascontiguousarray(Wk[gs, :].T).astype(bf16),
            "wvT": np.ascontiguousarray(Wv[gs, :].T).astype(bf16),
            "woT": np.ascontiguousarray(Wo[:, gs].T).astype(bf16),
            "bqp": np.ascontiguousarray(bq[gs].reshape(NPAIR, 128).T),
            "bkp": np.ascontiguousarray(bk[gs].reshape(NPAIR, 128).T),
            "ones1": ones1.astype(bf16),
            "cc": cc.astype(bf16), "ss": ss.astype(bf16),
        })
    return in_maps


def run(inputs: dict, trace: bool = False, tmpdir: str | None = None):
    """Returns (out [B, QL, DIM] f32, exec_time_ns or None)."""
    from concourse.bass_utils import run_bass_kernel_spmd

    nc = _build_nc()
    in_maps = _make_in_maps(**inputs)
    res = run_bass_kernel_spmd(nc, in_maps, list(range(NCORE)), trace=trace,
                               tmpdir=tmpdir)
    bo = np.asarray(inputs["bo"], np.float32)
    # bv was folded out of the device kernel: attn rows carry (v_proj)·p
    # only, and sum(p) = 1, so the bias contributes Wo @ bv to every row.
    bo = bo + np.asarray(inputs["Wo"], np.float32) @ np.asarray(
        inputs["bv"], np.float32)
    outs = [res.results[i]["out"] for i in range(NCORE)]
    out = np.stack([outs[G * b] + outs[G * b + 1] for b in range(B)])
    out += bo[None, None, :]
    return out.astype(np.float32), res.exec_time_ns


def kernel(**inputs) -> np.ndarray:
    out, _ = run(inputs, trace=False)
    return out
